# revision 25
# baseline (speedup 1.0000x reference)
"""Trainium2 Bass kernel for the ARqGPSFull autoregressive wavefunction.

Math: out[b] = sum_{s,m} ctx[b,s,m] * I_{x[b,s]}[s,m], with
ctx[b,s,m] = prod_{j<max(s,1)} P[s, x[b,j], m, j].  Taking logs turns the
masked product into an affine function of x:
  log ctx = C[s,m] + sum_j x[b,j]*D[s,m,j],   D = mask*(log P1 - log P0)
Folding |I_d|, arg(I_d) into exponent/phase AND folding the observed-state
selection d = x[b,s] into the same contraction via diagonal entries
  D'[s,m,j] = D[s,m,j] + delta_{j,s} * (log|I1/I0|, arg(I1/I0))[s,m]
gives a single complex exponent per (b,s,m):
  T[b,s,m] = exp(Sre + CA[s,m]) * cis(Sim + PH[s,m]),   out[b] = sum T
Each core owns 8 of the 64 m-values: one bf16 matmul
(66x128 stationary x-0.5 with two ones rows, 66x1024 moving params; 2 blocks
of 512 columns: Sre | Sim; rows 64/65 carry the per-column constant split
hi/lo in bf16).  exp comes from tanh, which shares an activation-table set
with Sin (one ~2.7us table load): -exp(S) = 2/(tanh(S/2)-1) + 1.
sin/cos use Sin (valid only on [-pi,pi]) with branchless range reduction
sin(t) = -sin(t - pi*sign(t)) for |t| < 2pi; the sign flips cancel between
the negated exp and negated trig.  Full-row reductions produce per-sample
partials; the host sums the 8 cores and wraps the phase.
"""

import sys

for _p in ("/opt/trn_rl_repo", "/root/.axon_site/_ro/trn_rl_repo"):
    if _p not in sys.path:
        sys.path.append(_p)

import math
import numpy as np
import ml_dtypes

N_CORES = 8
B = 128        # batch
L = 64         # n_sites
M = 64         # GPS support dim
NM = M // N_CORES   # m-values per core
NBLK = L * NM  # 512 columns per block
PI = math.pi

_BF16 = ml_dtypes.bfloat16

_built = None


def _build():
    import concourse.bacc as bacc
    import concourse.mybir as mybir
    from concourse import tile

    f32 = mybir.dt.float32
    bf16 = mybir.dt.bfloat16
    AF = mybir.ActivationFunctionType
    ALU = mybir.AluOpType
    AX = mybir.AxisListType

    nc = bacc.Bacc()
    # one (66, 1152) bf16 input: two 512-col param blocks (rows j=0..63 +
    # const hi/lo rows 64/65), then xt (x-0.5 transposed + two ones rows)
    rhs_d = nc.dram_tensor("rhs", [66, 2 * NBLK + B], bf16,
                           kind="ExternalInput")
    out_d = nc.dram_tensor("out", [B, 2], f32, kind="ExternalOutput")

    with tile.TileContext(nc) as tc:
        with (
            tc.tile_pool(name="pc", bufs=1) as pc,
            tc.tile_pool(name="psum", bufs=1, space="PSUM") as psum,
        ):
            rhs = pc.tile([66, 2 * NBLK + B], bf16, tag="rhs")
            nc.sync.dma_start(rhs[:], rhs_d[:])
            xt = rhs[:, 2 * NBLK:2 * NBLK + B]
            bm1 = pc.tile([B, 1], f32, tag="bm1")
            nc.gpsimd.memset(bm1[:], -1.0)
            bp1 = pc.tile([B, 1], f32, tag="bp1")
            nc.gpsimd.memset(bp1[:], 1.0)

            S = psum.tile([B, 2 * NBLK], f32, tag="S")
            nc.tensor.matmul(S[:, NBLK:2 * NBLK], xt,
                             rhs[:, NBLK:2 * NBLK], start=True, stop=True)
            nc.tensor.matmul(S[:, 0:NBLK], xt, rhs[:, 0:NBLK],
                             start=True, stop=True)
            S0 = S[:, 0:NBLK]
            S1 = S[:, NBLK:2 * NBLK]

            # negated sin/cos with Sign-based range reduction (longest chain
            # first so ACT starts it as soon as S1 lands)
            gs = pc.tile([B, NBLK], f32, tag="t_gs")
            nc.scalar.activation(gs[:], S1, AF.Sign)
            ths = pc.tile([B, NBLK], f32, tag="t_ths")
            nc.vector.scalar_tensor_tensor(
                ths[:], gs[:], -PI, S1, op0=ALU.mult, op1=ALU.add
            )
            sn = pc.tile([B, NBLK], f32, tag="sn")
            nc.scalar.activation(sn[:], ths[:], AF.Sin)
            # cos from the reduced sin argument: thc = ths + pi/2 - 2pi*[ths>pi/2]
            w2 = pc.tile([B, NBLK], f32, tag="t_w2")
            nc.vector.tensor_scalar(w2[:], ths[:], PI / 2, -2 * PI,
                                    op0=ALU.is_gt, op1=ALU.mult)
            thc = pc.tile([B, NBLK], f32, tag="t_thc")
            nc.vector.scalar_tensor_tensor(
                thc[:], ths[:], PI / 2, w2[:], op0=ALU.add, op1=ALU.add
            )
            cn = pc.tile([B, NBLK], f32, tag="cn")
            nc.scalar.activation(cn[:], thc[:], AF.Sin)

            # negated exp via tanh: en = -exp(S0) = 2/(tanh(S0/2)-1) + 1
            t = pc.tile([B, NBLK], f32, tag="t_tanh")
            nc.scalar.activation(t[:], S0, AF.Tanh, scale=0.5)
            den = pc.tile([B, NBLK], f32, tag="t_den")
            nc.scalar.activation(den[:], t[:], AF.Identity, bias=bm1[:])
            r = pc.tile([B, NBLK], f32, tag="t_r")
            nc.vector.reciprocal_approx_fast(r[:], den[:])
            en = pc.tile([B, NBLK], f32, tag="en")
            nc.scalar.activation(en[:], r[:], AF.Identity, bias=bp1[:],
                                 scale=2.0)

            # products (negations cancel) and full-row reductions; both
            # reduces land in one (B,2) tile so a single DMA writes out
            import os as _os
            o = pc.tile([B, 2], f32, tag="o")
            if _os.environ.get("USE_TTR") == "1":
                Tim = pc.tile([B, NBLK], f32, tag="Tim")
                nc.vector.tensor_tensor_reduce(
                    out=Tim[:], in0=en[:], in1=sn[:], scale=1.0, scalar=0.0,
                    op0=ALU.mult, op1=ALU.add, accum_out=o[:, 1:2],
                )
                Tre = pc.tile([B, NBLK], f32, tag="Tre")
                nc.vector.tensor_tensor_reduce(
                    out=Tre[:], in0=en[:], in1=cn[:], scale=1.0, scalar=0.0,
                    op0=ALU.mult, op1=ALU.add, accum_out=o[:, 0:1],
                )
            else:
                Tim = pc.tile([B, NBLK], f32, tag="Tim")
                nc.vector.tensor_mul(Tim[:], en[:], sn[:])
                nc.vector.tensor_reduce(o[:, 1:2], Tim[:], axis=AX.X,
                                        op=ALU.add)
                Tre = pc.tile([B, NBLK], f32, tag="Tre")
                nc.vector.tensor_mul(Tre[:], en[:], cn[:])
                nc.vector.tensor_reduce(o[:, 0:1], Tre[:], axis=AX.X,
                                        op=ALU.add)
            nc.sync.dma_start(out_d[:], o[:])

    nc.compile()

    # Bacc's act-table chooser is per-function greedy and picks two sets
    # (exp_and_others for Tanh, trig_and_small for Sin); silu_and_others
    # contains Tanh+Sin+Sign, so one ~2.7us load suffices.  The loads are
    # emitted sync-free, so retargeting the first and dropping the rest
    # is safe.
    from concourse.hw_specs import get_activation_tables
    silu_id = list(get_activation_tables(nc.m.arch)).index("silu_and_others")
    for blk in nc.main_func.blocks:
        loads = [i for i in blk.instructions
                 if type(i).__name__ == "InstLoadActFuncSet"]
        for n, ins in enumerate(loads):
            si = ins.sync_info
            assert si is None or (not si.on_wait and not si.on_update)
            if n == 0:
                ins.act_func_set_id = silu_id
            else:
                blk.instructions.remove(ins)

    # Hoist the (wait-free) input DMAs and the act-table load into the
    # preamble block, ahead of each engine's barrier participation: their
    # ~1us issue / ~1.3us table DMA then overlap the ~7us fixed preamble
    # instead of serializing after the tile entry barrier.  Their DMAHW
    # semaphore increments are consumed by waits that stay in the body,
    # and the kernel tail resets those semaphores for the next run.
    import os
    ET = mybir.EngineType
    b0, b1 = nc.main_func.blocks[0], nc.main_func.blocks[1]
    hoist = []
    if os.environ.get("NO_HOIST") == "1":
        return nc
    for ins in list(b1.instructions):
        nm = type(ins).__name__
        if nm == "InstLoadActFuncSet":
            hoist.append(ins)
            b1.instructions.remove(ins)
    for ins in hoist:
        first = next(i for i, x in enumerate(b0.instructions)
                     if x.engine == ins.engine)
        b0.instructions.insert(first, ins)
    return nc


def _host_pack(inputs, params_context, inputs_param):
    x = np.asarray(inputs).astype(np.float64)          # (B, L) in {0,1}
    P = np.asarray(params_context)                     # (s, d, m, j) complex
    I = np.asarray(inputs_param)                       # (s, d, m) complex

    mask = (np.arange(L)[None, :] < np.maximum(np.arange(L), 1)[:, None])
    Lp = np.log(P)
    D = (Lp[:, 1] - Lp[:, 0]) * mask[:, None, :]       # (s, m, j)
    C = (Lp[:, 0] * mask[:, None, :]).sum(-1)          # (s, m)
    I0 = I[:, 0]
    I1 = I[:, 1]
    A0 = np.log(np.abs(I0))
    dA = np.log(np.abs(I1)) - A0
    wrap = lambda t: np.angle(np.exp(1j * t))
    ph0 = np.angle(I0)
    dPh = wrap(np.angle(I1) - ph0)
    eye = np.eye(L)[:, None, :]                        # (s, 1, j)
    Dre = D.real + eye * dA[:, :, None]                # (s, m, j)
    Dim = D.imag + eye * dPh[:, :, None]
    CA = C.real + A0 + 0.5 * Dre.sum(-1)               # x-centering shift
    PH = wrap(C.imag + ph0 + 0.5 * Dim.sum(-1))

    xt = np.concatenate([(x - 0.5).T, np.ones((2, B))], 0)  # (66, B)
    rhs_list = []
    for k in range(N_CORES):
        msl = slice(k * NM, (k + 1) * NM)
        blocks = []
        for Dp, const in ((Dre, CA), (Dim, PH)):
            Dc = Dp[:, msl, :].transpose(2, 0, 1).reshape(L, NBLK)  # (j, s*m)
            cc = const[:, msl].reshape(NBLK)
            hi = cc.astype(_BF16).astype(np.float64)
            lo = cc - hi
            blocks.append(np.concatenate([Dc, hi[None], lo[None]], 0))
        full = np.concatenate([blocks[0], blocks[1], xt], 1)
        rhs_list.append(full.astype(_BF16))  # (66, 1152)
    return rhs_list


def kernel(inputs, params_context, inputs_param):
    global _built
    from concourse.bass_utils import run_bass_kernel_spmd

    if _built is None:
        _built = _build()
    nc = _built

    rhs_list = _host_pack(inputs, params_context, inputs_param)
    in_maps = [{"rhs": rhs_list[k]} for k in range(N_CORES)]
    res = run_bass_kernel_spmd(nc, in_maps, list(range(N_CORES)))

    re = np.zeros(B, np.float64)
    im = np.zeros(B, np.float64)
    for k in range(N_CORES):
        o = np.asarray(res.results[k]["out"], np.float64)
        re += o[:, 0]
        im += o[:, 1]
    return (re + 1j * np.angle(np.exp(1j * im))).astype(np.complex128)


# revision 26
# speedup vs baseline: 1.0619x; 1.0619x over previous
"""Trainium2 Bass kernel for the ARqGPSFull autoregressive wavefunction.

Math: out[b] = sum_{s,m} ctx[b,s,m] * I_{x[b,s]}[s,m], with
ctx[b,s,m] = prod_{j<max(s,1)} P[s, x[b,j], m, j].  Taking logs turns the
masked product into an affine function of x:
  log ctx = C[s,m] + sum_j x[b,j]*D[s,m,j],   D = mask*(log P1 - log P0)
Folding |I_d|, arg(I_d) into exponent/phase AND folding the observed-state
selection d = x[b,s] into the same contraction via diagonal entries
  D'[s,m,j] = D[s,m,j] + delta_{j,s} * (log|I1/I0|, arg(I1/I0))[s,m]
gives a single complex exponent per (b,s,m):
  T[b,s,m] = exp(Sre + CA[s,m]) * cis(Sim + PH[s,m]),   out[b] = sum T
Each core owns 8 of the 64 m-values: one bf16 matmul
(66x128 stationary x-0.5 with two ones rows, 66x1024 moving params; 2 blocks
of 512 columns: Sre | Sim; rows 64/65 carry the per-column constant split
hi/lo in bf16).  exp comes from tanh, which shares an activation-table set
with Sin (one ~2.7us table load): -exp(S) = 2/(tanh(S/2)-1) + 1.
sin/cos use Sin (valid only on [-pi,pi]) with branchless range reduction
sin(t) = -sin(t - pi*sign(t)) for |t| < 2pi; the sign flips cancel between
the negated exp and negated trig.  Full-row reductions produce per-sample
partials; the host sums the 8 cores and wraps the phase.
"""

import sys

for _p in ("/opt/trn_rl_repo", "/root/.axon_site/_ro/trn_rl_repo"):
    if _p not in sys.path:
        sys.path.append(_p)

import math
import numpy as np
import ml_dtypes

N_CORES = 8
B = 128        # batch
L = 64         # n_sites
M = 64         # GPS support dim
NM = M // N_CORES   # m-values per core
NBLK = L * NM  # 512 columns per block
PI = math.pi

_BF16 = ml_dtypes.bfloat16

_built = None


def _build():
    import concourse.bacc as bacc
    import concourse.mybir as mybir
    from concourse import tile

    f32 = mybir.dt.float32
    bf16 = mybir.dt.bfloat16
    AF = mybir.ActivationFunctionType
    ALU = mybir.AluOpType
    AX = mybir.AxisListType

    nc = bacc.Bacc()
    # one (66, 1152) bf16 input: two 512-col param blocks (rows j=0..63 +
    # const hi/lo rows 64/65), then xt (x-0.5 transposed + two ones rows)
    rhs_d = nc.dram_tensor("rhs", [66, 2 * NBLK + B], bf16,
                           kind="ExternalInput")
    out_d = nc.dram_tensor("out", [B, 2], f32, kind="ExternalOutput")

    with tile.TileContext(nc) as tc:
        with (
            tc.tile_pool(name="pc", bufs=1) as pc,
            tc.tile_pool(name="psum", bufs=1, space="PSUM") as psum,
        ):
            rhs = pc.tile([66, 2 * NBLK + B], bf16, tag="rhs")
            nc.sync.dma_start(rhs[:], rhs_d[:])
            xt = rhs[:, 2 * NBLK:2 * NBLK + B]
            bm1 = pc.tile([B, 1], f32, tag="bm1")
            nc.gpsimd.memset(bm1[:], -1.0)
            bp1 = pc.tile([B, 1], f32, tag="bp1")
            nc.gpsimd.memset(bp1[:], 1.0)

            S = psum.tile([B, 2 * NBLK], f32, tag="S")
            nc.tensor.matmul(S[:, NBLK:2 * NBLK], xt,
                             rhs[:, NBLK:2 * NBLK], start=True, stop=True)
            nc.tensor.matmul(S[:, 0:NBLK], xt, rhs[:, 0:NBLK],
                             start=True, stop=True)
            S0 = S[:, 0:NBLK]
            S1 = S[:, NBLK:2 * NBLK]

            # Per 256-col chunk: exp chain first (its DVE recip overlaps
            # the ACT trig work), trig chain second; two chunks interleave
            # across ACT/DVE so neither engine idles during the ping-pong.
            import os as _os
            o = pc.tile([B, 2], f32, tag="o")
            Tre = pc.tile([B, NBLK], f32, tag="Tre")
            Tim = pc.tile([B, NBLK], f32, tag="Tim")
            CH = 2
            W = NBLK // CH
            for ci in range(CH):
                c0 = ci * W
                c1 = (ci + 1) * W
                Sre = S[:, c0:c1]
                Sim = S[:, NBLK + c0:NBLK + c1]
                # negated exp via tanh: en = -exp(S) = 2/(tanh(S/2)-1) + 1
                t = pc.tile([B, W], f32, tag=f"t_tanh{ci}")
                nc.scalar.activation(t[:], Sre, AF.Tanh, scale=0.5)
                den = pc.tile([B, W], f32, tag=f"t_den{ci}")
                nc.scalar.activation(den[:], t[:], AF.Identity, bias=bm1[:])
                r = pc.tile([B, W], f32, tag=f"t_r{ci}")
                nc.vector.reciprocal_approx_fast(r[:], den[:])
                en = pc.tile([B, W], f32, tag=f"en{ci}")
                nc.scalar.activation(en[:], r[:], AF.Identity, bias=bp1[:],
                                     scale=2.0)
                # negated sin/cos with Sign-based range reduction
                gs = pc.tile([B, W], f32, tag=f"t_gs{ci}")
                nc.scalar.activation(gs[:], Sim, AF.Sign)
                ths = pc.tile([B, W], f32, tag=f"t_ths{ci}")
                nc.vector.scalar_tensor_tensor(
                    ths[:], gs[:], -PI, Sim, op0=ALU.mult, op1=ALU.add
                )
                sn = pc.tile([B, W], f32, tag=f"sn{ci}")
                nc.scalar.activation(sn[:], ths[:], AF.Sin)
                # cos from reduced arg: thc = ths + pi/2 - 2pi*[ths>pi/2]
                w2 = pc.tile([B, W], f32, tag=f"t_w2{ci}")
                nc.vector.tensor_scalar(w2[:], ths[:], PI / 2, -2 * PI,
                                        op0=ALU.is_gt, op1=ALU.mult)
                thc = pc.tile([B, W], f32, tag=f"t_thc{ci}")
                nc.vector.scalar_tensor_tensor(
                    thc[:], ths[:], PI / 2, w2[:], op0=ALU.add, op1=ALU.add
                )
                cn = pc.tile([B, W], f32, tag=f"cn{ci}")
                nc.scalar.activation(cn[:], thc[:], AF.Sin)
                # products (negations cancel)
                nc.vector.tensor_mul(Tim[:, c0:c1], en[:], sn[:])
                nc.vector.tensor_mul(Tre[:, c0:c1], en[:], cn[:])
            # full-row reductions; both land in one (B,2) tile, single DMA
            nc.vector.tensor_reduce(o[:, 1:2], Tim[:], axis=AX.X, op=ALU.add)
            nc.vector.tensor_reduce(o[:, 0:1], Tre[:], axis=AX.X, op=ALU.add)
            nc.sync.dma_start(out_d[:], o[:])

    nc.compile()

    # Bacc's act-table chooser is per-function greedy and picks two sets
    # (exp_and_others for Tanh, trig_and_small for Sin); silu_and_others
    # contains Tanh+Sin+Sign, so one ~2.7us load suffices.  The loads are
    # emitted sync-free, so retargeting the first and dropping the rest
    # is safe.
    from concourse.hw_specs import get_activation_tables
    silu_id = list(get_activation_tables(nc.m.arch)).index("silu_and_others")
    for blk in nc.main_func.blocks:
        loads = [i for i in blk.instructions
                 if type(i).__name__ == "InstLoadActFuncSet"]
        for n, ins in enumerate(loads):
            si = ins.sync_info
            assert si is None or (not si.on_wait and not si.on_update)
            if n == 0:
                ins.act_func_set_id = silu_id
            else:
                blk.instructions.remove(ins)

    # Hoist the (wait-free) input DMAs and the act-table load into the
    # preamble block, ahead of each engine's barrier participation: their
    # ~1us issue / ~1.3us table DMA then overlap the ~7us fixed preamble
    # instead of serializing after the tile entry barrier.  Their DMAHW
    # semaphore increments are consumed by waits that stay in the body,
    # and the kernel tail resets those semaphores for the next run.
    import os
    ET = mybir.EngineType
    b0, b1 = nc.main_func.blocks[0], nc.main_func.blocks[1]
    hoist = []
    if os.environ.get("NO_HOIST") == "1":
        return nc
    for ins in list(b1.instructions):
        nm = type(ins).__name__
        if nm == "InstDMACopy" and ins.engine == ET.SP:
            si = ins.sync_info
            if si is not None and si.on_wait:
                continue  # output DMA — depends on body results
            hoist.append(ins)
            b1.instructions.remove(ins)
        elif nm == "InstLoadActFuncSet":
            hoist.append(ins)
            b1.instructions.remove(ins)
    for ins in hoist:
        first = next(i for i, x in enumerate(b0.instructions)
                     if x.engine == ins.engine)
        b0.instructions.insert(first, ins)
    return nc


def _host_pack(inputs, params_context, inputs_param):
    x = np.asarray(inputs).astype(np.float64)          # (B, L) in {0,1}
    P = np.asarray(params_context)                     # (s, d, m, j) complex
    I = np.asarray(inputs_param)                       # (s, d, m) complex

    mask = (np.arange(L)[None, :] < np.maximum(np.arange(L), 1)[:, None])
    Lp = np.log(P)
    D = (Lp[:, 1] - Lp[:, 0]) * mask[:, None, :]       # (s, m, j)
    C = (Lp[:, 0] * mask[:, None, :]).sum(-1)          # (s, m)
    I0 = I[:, 0]
    I1 = I[:, 1]
    A0 = np.log(np.abs(I0))
    dA = np.log(np.abs(I1)) - A0
    wrap = lambda t: np.angle(np.exp(1j * t))
    ph0 = np.angle(I0)
    dPh = wrap(np.angle(I1) - ph0)
    eye = np.eye(L)[:, None, :]                        # (s, 1, j)
    Dre = D.real + eye * dA[:, :, None]                # (s, m, j)
    Dim = D.imag + eye * dPh[:, :, None]
    CA = C.real + A0 + 0.5 * Dre.sum(-1)               # x-centering shift
    PH = wrap(C.imag + ph0 + 0.5 * Dim.sum(-1))

    xt = np.concatenate([(x - 0.5).T, np.ones((2, B))], 0)  # (66, B)
    rhs_list = []
    for k in range(N_CORES):
        msl = slice(k * NM, (k + 1) * NM)
        blocks = []
        for Dp, const in ((Dre, CA), (Dim, PH)):
            Dc = Dp[:, msl, :].transpose(2, 0, 1).reshape(L, NBLK)  # (j, s*m)
            cc = const[:, msl].reshape(NBLK)
            hi = cc.astype(_BF16).astype(np.float64)
            lo = cc - hi
            blocks.append(np.concatenate([Dc, hi[None], lo[None]], 0))
        full = np.concatenate([blocks[0], blocks[1], xt], 1)
        rhs_list.append(full.astype(_BF16))  # (66, 1152)
    return rhs_list


def kernel(inputs, params_context, inputs_param):
    global _built
    from concourse.bass_utils import run_bass_kernel_spmd

    if _built is None:
        _built = _build()
    nc = _built

    rhs_list = _host_pack(inputs, params_context, inputs_param)
    in_maps = [{"rhs": rhs_list[k]} for k in range(N_CORES)]
    res = run_bass_kernel_spmd(nc, in_maps, list(range(N_CORES)))

    re = np.zeros(B, np.float64)
    im = np.zeros(B, np.float64)
    for k in range(N_CORES):
        o = np.asarray(res.results[k]["out"], np.float64)
        re += o[:, 0]
        im += o[:, 1]
    return (re + 1j * np.angle(np.exp(1j * im))).astype(np.complex128)


# revision 29
# speedup vs baseline: 1.0678x; 1.0055x over previous
"""Trainium2 Bass kernel for the ARqGPSFull autoregressive wavefunction.

Math: out[b] = sum_{s,m} ctx[b,s,m] * I_{x[b,s]}[s,m], with
ctx[b,s,m] = prod_{j<max(s,1)} P[s, x[b,j], m, j].  Taking logs turns the
masked product into an affine function of x:
  log ctx = C[s,m] + sum_j x[b,j]*D[s,m,j],   D = mask*(log P1 - log P0)
Folding |I_d|, arg(I_d) into exponent/phase AND folding the observed-state
selection d = x[b,s] into the same contraction via diagonal entries
  D'[s,m,j] = D[s,m,j] + delta_{j,s} * (log|I1/I0|, arg(I1/I0))[s,m]
gives a single complex exponent per (b,s,m):
  T[b,s,m] = exp(Sre + CA[s,m]) * cis(Sim + PH[s,m]),   out[b] = sum T
Each core owns 8 of the 64 m-values: one bf16 matmul
(66x128 stationary x-0.5 with two ones rows, 66x1024 moving params; 2 blocks
of 512 columns: Sre | Sim; rows 64/65 carry the per-column constant split
hi/lo in bf16).  exp comes from tanh, which shares an activation-table set
with Sin (one ~2.7us table load): -exp(S) = 2/(tanh(S/2)-1) + 1.
sin/cos use Sin (valid only on [-pi,pi]) with branchless range reduction
sin(t) = -sin(t - pi*sign(t)) for |t| < 2pi; the sign flips cancel between
the negated exp and negated trig.  Full-row reductions produce per-sample
partials; the host sums the 8 cores and wraps the phase.
"""

import sys

for _p in ("/opt/trn_rl_repo", "/root/.axon_site/_ro/trn_rl_repo"):
    if _p not in sys.path:
        sys.path.append(_p)

import math
import numpy as np
import ml_dtypes

N_CORES = 8
B = 128        # batch
L = 64         # n_sites
M = 64         # GPS support dim
NM = M // N_CORES   # m-values per core
NBLK = L * NM  # 512 columns per block
PI = math.pi

_BF16 = ml_dtypes.bfloat16

_built = None


def _build():
    import concourse.bacc as bacc
    import concourse.mybir as mybir
    from concourse import tile

    f32 = mybir.dt.float32
    bf16 = mybir.dt.bfloat16
    AF = mybir.ActivationFunctionType
    ALU = mybir.AluOpType
    AX = mybir.AxisListType

    import os
    use_tp = os.environ.get("USE_TP", "1") == "1"
    nc = bacc.Bacc()
    # one (66, 1152) bf16 input: two 512-col param blocks (rows j=0..63 +
    # const hi/lo rows 64/65), then xt (x-0.5 transposed + two ones rows)
    rhs_d = nc.dram_tensor("rhs", [66, 2 * NBLK + B], bf16,
                           kind="ExternalInput")
    if use_tp:
        ident_d = nc.dram_tensor("ident", [B, B], f32, kind="ExternalInput")
        out_d = nc.dram_tensor("out", [2, B], f32, kind="ExternalOutput")
    else:
        out_d = nc.dram_tensor("out", [B, 2], f32, kind="ExternalOutput")

    with tile.TileContext(nc) as tc:
        with (
            tc.tile_pool(name="pc", bufs=1) as pc,
            tc.tile_pool(name="psum", bufs=1, space="PSUM") as psum,
        ):
            rhs = pc.tile([66, 2 * NBLK + B], bf16, tag="rhs")
            nc.sync.dma_start(rhs[:], rhs_d[:])
            if use_tp:
                ident = pc.tile([B, B], f32, tag="ident")
                nc.sync.dma_start(ident[:], ident_d[:])
            xt = rhs[:, 2 * NBLK:2 * NBLK + B]
            bm1 = pc.tile([B, 1], f32, tag="bm1")
            nc.gpsimd.memset(bm1[:], -1.0)
            bp1 = pc.tile([B, 1], f32, tag="bp1")
            nc.gpsimd.memset(bp1[:], 1.0)

            S = psum.tile([B, 2 * NBLK], f32, tag="S")
            nc.tensor.matmul(S[:, NBLK:2 * NBLK], xt,
                             rhs[:, NBLK:2 * NBLK], start=True, stop=True)
            nc.tensor.matmul(S[:, 0:NBLK], xt, rhs[:, 0:NBLK],
                             start=True, stop=True)
            S0 = S[:, 0:NBLK]
            S1 = S[:, NBLK:2 * NBLK]

            # Per 256-col chunk: exp chain first (its DVE recip overlaps
            # the ACT trig work), trig chain second; two chunks interleave
            # across ACT/DVE so neither engine idles during the ping-pong.
            import os as _os
            o = pc.tile([B, 2], f32, tag="o")
            Tre = pc.tile([B, NBLK], f32, tag="Tre")
            Tim = pc.tile([B, NBLK], f32, tag="Tim")
            CH = 2
            W = NBLK // CH
            for ci in range(CH):
                c0 = ci * W
                c1 = (ci + 1) * W
                Sre = S[:, c0:c1]
                Sim = S[:, NBLK + c0:NBLK + c1]
                # negated exp via tanh: en = -exp(S) = 2/(tanh(S/2)-1) + 1
                t = pc.tile([B, W], f32, tag=f"t_tanh{ci}")
                nc.scalar.activation(t[:], Sre, AF.Tanh, scale=0.5)
                den = pc.tile([B, W], f32, tag=f"t_den{ci}")
                nc.scalar.activation(den[:], t[:], AF.Identity, bias=bm1[:])
                r = pc.tile([B, W], f32, tag=f"t_r{ci}")
                nc.vector.reciprocal_approx_fast(r[:], den[:])
                en = pc.tile([B, W], f32, tag=f"en{ci}")
                nc.scalar.activation(en[:], r[:], AF.Identity, bias=bp1[:],
                                     scale=2.0)
                # negated sin/cos with Sign-based range reduction
                gs = pc.tile([B, W], f32, tag=f"t_gs{ci}")
                nc.scalar.activation(gs[:], Sim, AF.Sign)
                ths = pc.tile([B, W], f32, tag=f"t_ths{ci}")
                nc.vector.scalar_tensor_tensor(
                    ths[:], gs[:], -PI, Sim, op0=ALU.mult, op1=ALU.add
                )
                sn = pc.tile([B, W], f32, tag=f"sn{ci}")
                nc.scalar.activation(sn[:], ths[:], AF.Sin)
                # cos from reduced arg: thc = ths + pi/2 - 2pi*[ths>pi/2]
                w2 = pc.tile([B, W], f32, tag=f"t_w2{ci}")
                nc.vector.tensor_scalar(w2[:], ths[:], PI / 2, -2 * PI,
                                        op0=ALU.is_gt, op1=ALU.mult)
                thc = pc.tile([B, W], f32, tag=f"t_thc{ci}")
                nc.vector.scalar_tensor_tensor(
                    thc[:], ths[:], PI / 2, w2[:], op0=ALU.add, op1=ALU.add
                )
                cn = pc.tile([B, W], f32, tag=f"cn{ci}")
                nc.scalar.activation(cn[:], thc[:], AF.Sin)
                # products (negations cancel)
                nc.vector.tensor_mul(Tim[:, c0:c1], en[:], sn[:])
                nc.vector.tensor_mul(Tre[:, c0:c1], en[:], cn[:])
            # full-row reductions; both land in one (B,2) tile
            nc.vector.tensor_reduce(o[:, 1:2], Tim[:], axis=AX.X, op=ALU.add)
            nc.vector.tensor_reduce(o[:, 0:1], Tre[:], axis=AX.X, op=ALU.add)
            if use_tp:
                # transpose (128,2) -> (2,128) on PE so the output DMA is
                # two contiguous rows, not 128 tiny strided descriptors
                ot = psum.tile([2, B], f32, tag="ot")
                nc.tensor.matmul(ot[:], o[:], ident[:], start=True, stop=True)
                ots = pc.tile([2, B], f32, tag="ots")
                nc.vector.tensor_copy(ots[:], ot[:])
                nc.sync.dma_start(out_d[:], ots[:])
            else:
                nc.sync.dma_start(out_d[:], o[:])

    nc.compile()

    # Bacc's act-table chooser is per-function greedy and picks two sets
    # (exp_and_others for Tanh, trig_and_small for Sin); silu_and_others
    # contains Tanh+Sin+Sign, so one ~2.7us load suffices.  The loads are
    # emitted sync-free, so retargeting the first and dropping the rest
    # is safe.
    from concourse.hw_specs import get_activation_tables
    silu_id = list(get_activation_tables(nc.m.arch)).index("silu_and_others")
    for blk in nc.main_func.blocks:
        loads = [i for i in blk.instructions
                 if type(i).__name__ == "InstLoadActFuncSet"]
        for n, ins in enumerate(loads):
            si = ins.sync_info
            assert si is None or (not si.on_wait and not si.on_update)
            if n == 0:
                ins.act_func_set_id = silu_id
            else:
                blk.instructions.remove(ins)

    # Hoist the (wait-free) input DMAs and the act-table load into the
    # preamble block, ahead of each engine's barrier participation: their
    # ~1us issue / ~1.3us table DMA then overlap the ~7us fixed preamble
    # instead of serializing after the tile entry barrier.  Their DMAHW
    # semaphore increments are consumed by waits that stay in the body,
    # and the kernel tail resets those semaphores for the next run.
    import os
    ET = mybir.EngineType
    b0, b1 = nc.main_func.blocks[0], nc.main_func.blocks[1]
    hoist = []
    if os.environ.get("NO_HOIST") == "1":
        return nc
    for ins in list(b1.instructions):
        nm = type(ins).__name__
        if nm == "InstDMACopy" and ins.engine == ET.SP:
            si = ins.sync_info
            if si is not None and si.on_wait:
                continue  # output DMA — depends on body results
            hoist.append(ins)
            b1.instructions.remove(ins)
        elif nm == "InstLoadActFuncSet":
            hoist.append(ins)
            b1.instructions.remove(ins)
    for ins in hoist:
        first = next(i for i, x in enumerate(b0.instructions)
                     if x.engine == ins.engine)
        b0.instructions.insert(first, ins)
    return nc


def _host_pack(inputs, params_context, inputs_param):
    x = np.asarray(inputs).astype(np.float64)          # (B, L) in {0,1}
    P = np.asarray(params_context)                     # (s, d, m, j) complex
    I = np.asarray(inputs_param)                       # (s, d, m) complex

    mask = (np.arange(L)[None, :] < np.maximum(np.arange(L), 1)[:, None])
    Lp = np.log(P)
    D = (Lp[:, 1] - Lp[:, 0]) * mask[:, None, :]       # (s, m, j)
    C = (Lp[:, 0] * mask[:, None, :]).sum(-1)          # (s, m)
    I0 = I[:, 0]
    I1 = I[:, 1]
    A0 = np.log(np.abs(I0))
    dA = np.log(np.abs(I1)) - A0
    wrap = lambda t: np.angle(np.exp(1j * t))
    ph0 = np.angle(I0)
    dPh = wrap(np.angle(I1) - ph0)
    eye = np.eye(L)[:, None, :]                        # (s, 1, j)
    Dre = D.real + eye * dA[:, :, None]                # (s, m, j)
    Dim = D.imag + eye * dPh[:, :, None]
    CA = C.real + A0 + 0.5 * Dre.sum(-1)               # x-centering shift
    PH = wrap(C.imag + ph0 + 0.5 * Dim.sum(-1))

    xt = np.concatenate([(x - 0.5).T, np.ones((2, B))], 0)  # (66, B)
    rhs_list = []
    for k in range(N_CORES):
        msl = slice(k * NM, (k + 1) * NM)
        blocks = []
        for Dp, const in ((Dre, CA), (Dim, PH)):
            Dc = Dp[:, msl, :].transpose(2, 0, 1).reshape(L, NBLK)  # (j, s*m)
            cc = const[:, msl].reshape(NBLK)
            hi = cc.astype(_BF16).astype(np.float64)
            lo = cc - hi
            blocks.append(np.concatenate([Dc, hi[None], lo[None]], 0))
        full = np.concatenate([blocks[0], blocks[1], xt], 1)
        rhs_list.append(full.astype(_BF16))  # (66, 1152)
    return rhs_list


def kernel(inputs, params_context, inputs_param):
    global _built
    from concourse.bass_utils import run_bass_kernel_spmd

    if _built is None:
        _built = _build()
    nc = _built

    import os
    rhs_list = _host_pack(inputs, params_context, inputs_param)
    if os.environ.get("USE_TP", "1") == "1":
        ident = np.eye(B, dtype=np.float32)
        in_maps = [{"rhs": rhs_list[k], "ident": ident}
                   for k in range(N_CORES)]
    else:
        in_maps = [{"rhs": rhs_list[k]} for k in range(N_CORES)]
    res = run_bass_kernel_spmd(nc, in_maps, list(range(N_CORES)))

    re = np.zeros(B, np.float64)
    im = np.zeros(B, np.float64)
    for k in range(N_CORES):
        o = np.asarray(res.results[k]["out"], np.float64)
        if o.shape[0] == 2:
            re += o[0]
            im += o[1]
        else:
            re += o[:, 0]
            im += o[:, 1]
    return (re + 1j * np.angle(np.exp(1j * im))).astype(np.complex128)
